# revision 14
# baseline (speedup 1.0000x reference)
"""DotGATHead Trainium2 kernel: LN -> 4-head masked attention -> MLP (2x swish+LN+linear) -> LN.

Sharding: query rows. 8 cores = 4 batches x 2 query-halves. Each core gets its
batch's x (rolled so its query rows are always rows 0..1023 -> one SPMD program),
computes K/V over all 2048 nodes, attention + MLP for its 1024 queries.

Layouts: "T layout" = features on partitions, nodes on free dim.
  xnT/KT/QT computed via LN (row layout) + PE transpose / projection matmuls.
  scores^T computed per (ktile, qchunk) into PSUM, exp on ACT -> bf16 expTm,
  mask multiply on DVE, attnV with a ones column appended to V giving row-layout
  output AND softmax denominators in one matmul group (no max-subtraction:
  LN-bounded scores keep exp() in fp32/bf16 range; verified in test harness).
MLP folds ln1/ln2 weights into fc matrices (host-side), standardization of ln2
via K=1 augmentation matmuls, biases via ACT activation bias / augmentation.
Matmuls run in float32r (TF32-like, 4x fp32 throughput), attention matrix bf16.
"""

import numpy as np
import ml_dtypes

import concourse.bass as bass
import concourse.mybir as mybir
import concourse.tile as tile
from concourse import bacc
from concourse.bass_utils import run_bass_kernel_spmd
from concourse.masks import make_identity

B, A, D, HEADS, HD = 4, 2048, 512, 4, 128
P = 128
QLOC = 1024              # query rows per core
NQS = QLOC // P          # 8 query subtiles
NKT = A // P             # 16 key tiles
NEO = D // P             # 4 feature partition-tiles
NQC = QLOC // 512        # 2 query chunks of 512
EPS = 1e-5
F32 = mybir.dt.float32
F32R = mybir.dt.float32r
BF16 = mybir.dt.bfloat16
FP16 = mybir.dt.float16
AF = mybir.ActivationFunctionType
OP = mybir.AluOpType

_compiled = {}


def _row_layernorm(nc, pool, xt, eps_t, tag=""):
    """In-place LN (standardize only) of row tile xt [P, D]."""
    st6 = pool.tile([P, 6], F32, tag="st6" + tag)
    nc.vector.bn_stats(out=st6[:], in_=xt[:])
    mv = pool.tile([P, 2], F32, tag="mv" + tag)
    nc.vector.bn_aggr(out=mv[:], in_=st6[:])
    sig = pool.tile([P, 1], F32, tag="sig" + tag)
    nc.scalar.activation(out=sig[:], in_=mv[:, 1:2], func=AF.Sqrt,
                         bias=eps_t[:], scale=1.0)
    rstd = pool.tile([P, 1], F32, tag="rstd" + tag)
    nc.vector.reciprocal(out=rstd[:], in_=sig[:])
    nc.vector.tensor_scalar(out=xt[:], in0=xt[:],
                            scalar1=mv[:, 0:1], scalar2=rstd[:],
                            op0=OP.subtract, op1=OP.mult)


def _build(use_v_bias, use_qk_bias, use_final_affine):
    nc = bacc.Bacc("TRN2", target_bir_lowering=False, debug=False, num_devices=8)

    x_d = nc.dram_tensor("x", [A, D], F32, kind="ExternalInput")
    mask_d = nc.dram_tensor("maskT", [P, NKT, QLOC], FP16, kind="ExternalInput")
    wq_d = nc.dram_tensor("wq_t", [D, D], F32R, kind="ExternalInput")
    wk_d = nc.dram_tensor("wk_t", [D, D], F32R, kind="ExternalInput")
    wv_d = nc.dram_tensor("wv_t", [D, D], F32R, kind="ExternalInput")
    fc1_d = nc.dram_tensor("fc1_t", [D, D], F32R, kind="ExternalInput")
    fc2_d = nc.dram_tensor("fc2_t", [D, D], F32R, kind="ExternalInput")
    fc1b_d = nc.dram_tensor("fc1b", [P, NEO], F32, kind="ExternalInput")
    fc2sum_d = nc.dram_tensor("fc2sum", [1, D], F32R, kind="ExternalInput")
    fc2b_d = nc.dram_tensor("fc2b", [1, D], F32R, kind="ExternalInput")
    qkvb_d = nc.dram_tensor("qkvb", [P, 3 * NEO], F32, kind="ExternalInput")
    naff_d = nc.dram_tensor("naff", [1, 2 * D], F32, kind="ExternalInput")
    vbrow_d = nc.dram_tensor("vbrow", [1, D], F32, kind="ExternalInput")
    onesd_d = nc.dram_tensor("onesd", [P, P], F32R, kind="ExternalInput")
    fc1sum_d = nc.dram_tensor("fc1sum", [1, D], F32R, kind="ExternalInput")
    fc1brow_d = nc.dram_tensor("fc1brow", [1, D], F32R, kind="ExternalInput")
    y_d = nc.dram_tensor("y", [QLOC, D], F32, kind="ExternalOutput")

    with tile.TileContext(nc) as tc:
        with tc.tile_pool(name="const", bufs=1) as const, \
             tc.tile_pool(name="orow", bufs=1) as orow:
            fc1 = const.tile([P, NEO, D], F32R)
            fc2 = const.tile([P, NEO, D], F32R)
            fc1b = const.tile([P, NEO], F32)
            nc.sync.dma_start(out=fc1b[:], in_=fc1b_d[:])
            fc2sum = const.tile([1, D], F32R)
            nc.sync.dma_start(out=fc2sum[:], in_=fc2sum_d[:])
            fc2b = const.tile([1, D], F32R)
            nc.sync.dma_start(out=fc2b[:], in_=fc2b_d[:])
            qkvb = const.tile([P, 3 * NEO], F32)
            if use_qk_bias or use_v_bias:
                nc.sync.dma_start(out=qkvb[:], in_=qkvb_d[:])
            vb_rep = const.tile([P, D], F32)
            if use_v_bias:
                vb_ap = vbrow_d[:, :]
                nc.gpsimd.dma_start(out=vb_rep[:], in_=bass.AP(
                    tensor=vb_ap.tensor, offset=vb_ap.offset,
                    ap=[[0, P], [1, D]]))
            naff = const.tile([P, 2 * D], F32)
            if use_final_affine:
                naff_ap = naff_d[:, :]
                nc.gpsimd.dma_start(out=naff[:], in_=bass.AP(
                    tensor=naff_ap.tensor, offset=naff_ap.offset,
                    ap=[[0, P], [1, 2 * D]]))
            ident = const.tile([P, P], F32)
            make_identity(nc, ident[:])
            eps_t = const.tile([P, 1], F32)
            nc.vector.memset(eps_t[:], EPS)
            ones_inv = const.tile([P, P], F32R)
            nc.sync.dma_start(out=ones_inv[:], in_=onesd_d[:, :])
            ones_h = const.tile([P, P], FP16)
            nc.vector.memset(ones_h[:], 1.0)
            fc1sum = const.tile([1, D], F32R)
            nc.sync.dma_start(out=fc1sum[:], in_=fc1sum_d[:])
            fc1brow = const.tile([1, D], F32R)
            nc.sync.dma_start(out=fc1brow[:], in_=fc1brow_d[:])

            outT = orow.tile([P, NEO, QLOC], F32)   # attention output, T layout [d, h, q]

            # ======== attention weights/data pools (closed before MLP) ========
            with tc.tile_pool(name="attw", bufs=1) as attw, \
                 tc.tile_pool(name="sps", bufs=2, space="PSUM") as sps:
                KT = attw.tile([P, HEADS, A], FP16)       # K^T/sqrt(hd): [d, h, node]
                QT = attw.tile([P, HEADS, QLOC], FP16)    # Q^T: [d, h, q]
                Vg = attw.tile([P, NKT, D], FP16)         # V rows [node, f]
                maskT = attw.tile([P, NKT, QLOC], FP16)
                nc.sync.dma_start(out=maskT[:], in_=mask_d[:])

                # ---- P1: LN(x) row tiles + transpose -> xnT; P2: projections ----
                with tc.tile_pool(name="xnp", bufs=1) as xnp, \
                     tc.tile_pool(name="p1t", bufs=6) as p1t, \
                     tc.tile_pool(name="p1ps", bufs=2, space="PSUM") as p1ps, \
                     tc.tile_pool(name="p2ps", bufs=2, space="PSUM") as p2ps:
                    xnT = xnp.tile([P, NEO, A], F32R)
                    wq = xnp.tile([P, NEO, D], F32R)
                    wk = xnp.tile([P, NEO, D], F32R)
                    wv = xnp.tile([P, NEO, D], F32R)
                    for r in range(NKT):
                        xt = p1t.tile([P, D], F32, tag="xt")
                        nc.sync.dma_start(out=xt[:], in_=x_d[r * P:(r + 1) * P, :])
                        _row_layernorm(nc, p1t, xt, eps_t)
                        tp = p1ps.tile([P, NEO, P], F32, tag="tp1")
                        for eo in range(NEO):
                            nc.tensor.transpose(tp[:, eo, :], xt[:, eo * P:(eo + 1) * P], ident[:])
                        nc.vector.tensor_copy(out=xnT[:, :, r * P:(r + 1) * P], in_=tp[:])

                    for t, d in ((wk, wk_d), (wq, wq_d), (wv, wv_d)):
                        nc.sync.dma_start(out=t[:], in_=d.rearrange("(eo p) f -> p eo f", p=P))
                    # KT (scaled by 1/sqrt(hd) via host-prescaled wk)
                    for h in range(HEADS):
                        for c in range(A // 512):
                            ps = p2ps.tile([P, 512], F32, tag="ps")
                            for eo in range(NEO):
                                nc.tensor.matmul(ps[:], wk[:, eo, h * HD:(h + 1) * HD],
                                                 xnT[:, eo, c * 512:(c + 1) * 512],
                                                 start=(eo == 0), stop=(eo == NEO - 1))
                            if use_qk_bias:
                                nc.scalar.activation(out=KT[:, h, c * 512:(c + 1) * 512],
                                                     in_=ps[:], func=AF.Copy,
                                                     bias=qkvb[:, NEO + h:NEO + h + 1],
                                                     scale=1.0)
                            else:
                                nc.scalar.copy(out=KT[:, h, c * 512:(c + 1) * 512], in_=ps[:])
                    # QT (queries = rolled nodes 0..QLOC-1)
                    for h in range(HEADS):
                        for c in range(NQC):
                            ps = p2ps.tile([P, 512], F32, tag="ps")
                            for eo in range(NEO):
                                nc.tensor.matmul(ps[:], wq[:, eo, h * HD:(h + 1) * HD],
                                                 xnT[:, eo, c * 512:(c + 1) * 512],
                                                 start=(eo == 0), stop=(eo == NEO - 1))
                            if use_qk_bias:
                                nc.scalar.activation(out=QT[:, h, c * 512:(c + 1) * 512],
                                                     in_=ps[:], func=AF.Copy,
                                                     bias=qkvb[:, h:h + 1], scale=1.0)
                            else:
                                nc.vector.tensor_copy(out=QT[:, h, c * 512:(c + 1) * 512],
                                                      in_=ps[:])
                    # V natural rows [node, f] in fp16
                    for kt in range(NKT):
                        ps = p2ps.tile([P, D], F32, tag="ps")
                        for eo in range(NEO):
                            nc.tensor.matmul(ps[:], xnT[:, eo, kt * P:(kt + 1) * P],
                                             wv[:, eo, :],
                                             start=(eo == 0), stop=(eo == NEO - 1))
                        if use_v_bias:
                            nc.vector.tensor_tensor(out=ps[:], in0=ps[:],
                                                    in1=vb_rep[:], op=OP.add)
                        nc.vector.tensor_copy(out=Vg[:, kt, :], in_=ps[:])

                # ---- P3: attention, head by head ----
                with tc.tile_pool(name="expp", bufs=2) as expp, \
                     tc.tile_pool(name="recp", bufs=2) as recp, \
                     tc.tile_pool(name="ops", bufs=2, space="PSUM") as ops:
                    for h in range(HEADS):
                        expTm = expp.tile([P, NKT, QLOC], FP16, tag="expTm")
                        # scores^T in kt-pairs, exp over [P, 2, 512] at once
                        for c in range(NQC):
                            for kp in range(NKT // 2):
                                ps = sps.tile([P, 2, 512], F32, tag="sc")
                                for j in range(2):
                                    kt = kp * 2 + j
                                    nc.tensor.matmul(ps[:, j, :],
                                                     KT[:, h, kt * P:(kt + 1) * P],
                                                     QT[:, h, c * 512:(c + 1) * 512],
                                                     start=True, stop=True)
                                nc.scalar.activation(
                                    out=expTm[:, kp * 2:kp * 2 + 2, c * 512:(c + 1) * 512],
                                    in_=ps[:], func=AF.Exp)
                        # mask multiply (quarters, to release attnV/sums early)
                        for qtr in range(4):
                            sl = slice(qtr * (NKT // 4), (qtr + 1) * (NKT // 4))
                            nc.vector.tensor_tensor(out=expTm[:, sl, :], in0=expTm[:, sl, :],
                                                    in1=maskT[:, sl, :], op=OP.mult)
                        # softmax denominators, replicated across partitions
                        rec_rep = recp.tile([P, QLOC], F32, tag="rec")
                        for c in range(NQC):
                            pss = ops.tile([P, 512], F32, tag="sums")
                            for kt in range(NKT):
                                nc.tensor.matmul(pss[:], ones_h[:],
                                                 expTm[:, kt, c * 512:(c + 1) * 512],
                                                 start=(kt == 0), stop=(kt == NKT - 1))
                            nc.vector.reciprocal_approx_fast(out=rec_rep[:, c * 512:(c + 1) * 512],
                                                             in_=pss[:])
                        # attnV -> out^T (T layout), normalize on copyback
                        for c in range(NQC):
                            po = ops.tile([P, 512], F32, tag="attnps")
                            for kt in range(NKT):
                                nc.tensor.matmul(po[:], Vg[:, kt, h * HD:(h + 1) * HD],
                                                 expTm[:, kt, c * 512:(c + 1) * 512],
                                                 start=(kt == 0), stop=(kt == NKT - 1))
                            nc.vector.tensor_tensor(out=outT[:, h, c * 512:(c + 1) * 512],
                                                    in0=po[:],
                                                    in1=rec_rep[:, c * 512:(c + 1) * 512],
                                                    op=OP.mult)

            # ======== P4: MLP ========
            with tc.tile_pool(name="mlp", bufs=1) as mlp, \
                 tc.tile_pool(name="p4t", bufs=4) as p4t, \
                 tc.tile_pool(name="fps", bufs=4, space="PSUM") as fps, \
                 tc.tile_pool(name="stps", bufs=1, space="PSUM") as stps:
                for t, d in ((fc1, fc1_d), (fc2, fc2_d)):
                    nc.sync.dma_start(out=t[:], in_=d.rearrange("(eo p) f -> p eo f", p=P))
                # swish(out) then LN1 stats in T layout (replicated ones-matmuls)
                s0T = mlp.tile([P, NEO, QLOC], F32R)
                sq1T = mlp.tile([P, NEO, QLOC], F32R)
                mu1 = mlp.tile([P, QLOC], F32)
                e1 = mlp.tile([P, QLOC], F32)
                var1 = mlp.tile([P, QLOC], F32)
                sig1 = mlp.tile([P, QLOC], F32R)
                rstd1 = mlp.tile([P, QLOC], F32)
                nmu1 = mlp.tile([P, QLOC], F32R)
                for c in range(NQC):
                    qsl = slice(c * 512, (c + 1) * 512)
                    nc.scalar.activation(out=s0T[:, :, qsl], in_=outT[:, :, qsl], func=AF.Silu)
                    nc.scalar.activation(out=sq1T[:, :, qsl], in_=s0T[:, :, qsl], func=AF.Square)
                    psm = stps.tile([P, 512], F32, tag="psmu")
                    for eo in range(NEO):
                        nc.tensor.matmul(psm[:], ones_inv[:], s0T[:, eo, qsl],
                                         start=(eo == 0), stop=(eo == NEO - 1))
                    nc.vector.tensor_copy(out=mu1[:, qsl], in_=psm[:])
                    pse = stps.tile([P, 512], F32, tag="pse2")
                    for eo in range(NEO):
                        nc.tensor.matmul(pse[:], ones_inv[:], sq1T[:, eo, qsl],
                                         start=(eo == 0), stop=(eo == NEO - 1))
                    nc.vector.tensor_copy(out=e1[:, qsl], in_=pse[:])
                    nc.vector.tensor_tensor(out=var1[:, qsl], in0=mu1[:, qsl],
                                            in1=mu1[:, qsl], op=OP.mult)
                    nc.vector.tensor_tensor(out=var1[:, qsl], in0=e1[:, qsl],
                                            in1=var1[:, qsl], op=OP.subtract)
                    nc.scalar.activation(out=sig1[:, qsl], in_=var1[:, qsl], func=AF.Sqrt,
                                         bias=eps_t[:], scale=1.0)
                    nc.vector.reciprocal_approx_fast(out=rstd1[:, qsl],
                                                     in_=sig1[:, qsl].bitcast(F32))
                    nc.vector.tensor_scalar(out=nmu1[:, qsl], in0=mu1[:, qsl], scalar1=-1.0,
                                            scalar2=None, op0=OP.mult)

                # fc1 with ln1 standardization + bias folded in; swish on copyback
                h1sT = mlp.tile([P, NEO, QLOC], F32R)
                for ft in range(NEO):
                    for c in range(NQC):
                        ps = fps.tile([P, 512], F32, tag="fcps")
                        for eo in range(NEO):
                            nc.tensor.matmul(ps[:], fc1[:, eo, ft * P:(ft + 1) * P],
                                             s0T[:, eo, c * 512:(c + 1) * 512],
                                             start=(eo == 0), stop=False)
                        nc.tensor.matmul(ps[:], fc1sum[:, ft * P:(ft + 1) * P],
                                         nmu1[0:1, c * 512:(c + 1) * 512],
                                         start=False, stop=False)
                        nc.tensor.matmul(ps[:], fc1brow[:, ft * P:(ft + 1) * P],
                                         sig1[0:1, c * 512:(c + 1) * 512],
                                         start=False, stop=True)
                        nc.vector.tensor_tensor(out=ps[:], in0=ps[:],
                                                in1=rstd1[:, c * 512:(c + 1) * 512],
                                                op=OP.mult)
                        nc.scalar.activation(out=h1sT[:, ft, c * 512:(c + 1) * 512],
                                             in_=ps[:], func=AF.Silu)

                # LN2 stats in T layout via replicated ones-matmuls
                sqT = mlp.tile([P, NEO, QLOC], F32R)
                mu2 = mlp.tile([P, QLOC], F32)
                e2 = mlp.tile([P, QLOC], F32)
                var2 = mlp.tile([P, QLOC], F32)
                sig2 = mlp.tile([P, QLOC], F32R)
                rstd2 = mlp.tile([P, QLOC], F32)
                nmu2 = mlp.tile([P, QLOC], F32R)
                for c in range(NQC):
                    qsl = slice(c * 512, (c + 1) * 512)
                    nc.scalar.activation(out=sqT[:, :, qsl], in_=h1sT[:, :, qsl], func=AF.Square)
                    psm = stps.tile([P, 512], F32, tag="psmu")
                    for eo in range(NEO):
                        nc.tensor.matmul(psm[:], ones_inv[:], h1sT[:, eo, qsl],
                                         start=(eo == 0), stop=(eo == NEO - 1))
                    nc.vector.tensor_copy(out=mu2[:, qsl], in_=psm[:])
                    pse = stps.tile([P, 512], F32, tag="pse2")
                    for eo in range(NEO):
                        nc.tensor.matmul(pse[:], ones_inv[:], sqT[:, eo, qsl],
                                         start=(eo == 0), stop=(eo == NEO - 1))
                    nc.vector.tensor_copy(out=e2[:, qsl], in_=pse[:])
                    nc.vector.tensor_tensor(out=var2[:, qsl], in0=mu2[:, qsl],
                                            in1=mu2[:, qsl], op=OP.mult)
                    nc.vector.tensor_tensor(out=var2[:, qsl], in0=e2[:, qsl],
                                            in1=var2[:, qsl], op=OP.subtract)
                    nc.scalar.activation(out=sig2[:, qsl], in_=var2[:, qsl], func=AF.Sqrt,
                                         bias=eps_t[:], scale=1.0)
                    nc.vector.reciprocal_approx_fast(out=rstd2[:, qsl],
                                                     in_=sig2[:, qsl].bitcast(F32))
                    nc.vector.tensor_scalar(out=nmu2[:, qsl], in0=mu2[:, qsl], scalar1=-1.0,
                                            scalar2=None, op0=OP.mult)

                # fc2 in ROW orientation (lhsT = h1sT tiles), ln2 standardization via
                # K=1 augmentation; per-row rstd2 applied via transposed stat columns.
                rstd2c = mlp.tile([P, NQS], F32)
                for g in range(2):
                    tpst = stps.tile([P, 4, P], F32, tag="tpstat")
                    for j in range(4):
                        qs = g * 4 + j
                        nc.tensor.transpose(tpst[:, j, :],
                                            rstd2[:, qs * P:(qs + 1) * P], ident[:])
                    nc.vector.tensor_copy(out=rstd2c[:, g * 4:(g + 1) * 4],
                                          in_=tpst[:, :, 0])
                for qs in range(NQS):
                    ps = fps.tile([P, 512], F32, tag="fcps")
                    for eo in range(NEO):
                        nc.tensor.matmul(ps[:], h1sT[:, eo, qs * P:(qs + 1) * P],
                                         fc2[:, eo, :],
                                         start=(eo == 0), stop=False)
                    nc.tensor.matmul(ps[:], nmu2[0:1, qs * P:(qs + 1) * P],
                                     fc2sum[:, :], start=False, stop=False)
                    nc.tensor.matmul(ps[:], sig2[0:1, qs * P:(qs + 1) * P],
                                     fc2b[:, :], start=False, stop=True)
                    hr = p4t.tile([P, D], F32, tag="hrow")
                    nc.vector.tensor_scalar(out=hr[:], in0=ps[:],
                                            scalar1=rstd2c[:, qs:qs + 1], scalar2=None,
                                            op0=OP.mult)
                    _row_layernorm(nc, p4t, hr, eps_t, tag="b")
                    if use_final_affine:
                        nc.vector.tensor_tensor(out=hr[:], in0=hr[:],
                                                in1=naff[:, 0:D], op=OP.mult)
                        nc.vector.tensor_tensor(out=hr[:], in0=hr[:],
                                                in1=naff[:, D:2 * D], op=OP.add)
                    nc.sync.dma_start(out=y_d[qs * P:(qs + 1) * P, :], in_=hr[:])

    nc.compile()
    return nc


def kernel(**inputs):
    x = np.asarray(inputs["x"], np.float32)
    conn = np.asarray(inputs["connectivity"])
    Wq = np.asarray(inputs["Wq"], np.float32)
    Wk = np.asarray(inputs["Wk"], np.float32)
    Wv = np.asarray(inputs["Wv"], np.float32)
    norm_w = np.asarray(inputs["norm_w"], np.float32)
    norm_b = np.asarray(inputs["norm_b"], np.float32)
    ln1_w = np.asarray(inputs["ln1_w"], np.float32)
    ln1_b = np.asarray(inputs["ln1_b"], np.float32)
    fc1_w = np.asarray(inputs["fc1_w"], np.float32)
    fc1_b = np.asarray(inputs["fc1_b"], np.float32)
    ln2_w = np.asarray(inputs["ln2_w"], np.float32)
    ln2_b = np.asarray(inputs["ln2_b"], np.float32)
    fc2_w = np.asarray(inputs["fc2_w"], np.float32)
    fc2_b = np.asarray(inputs["fc2_b"], np.float32)

    s = 1.0 / np.sqrt(HD)
    wq_t = np.ascontiguousarray(norm_w[:, None] * Wq.T)
    wk_t = np.ascontiguousarray((norm_w[:, None] * Wk.T) * np.float32(s))
    wv_t = np.ascontiguousarray(norm_w[:, None] * Wv.T)
    qb = Wq @ norm_b
    kb = (Wk @ norm_b) * s
    vb = Wv @ norm_b
    fc1_t = np.ascontiguousarray(ln1_w[:, None] * fc1_w.T)
    fc1b_eff = fc1_w @ ln1_b + fc1_b
    fc2_t = np.ascontiguousarray(ln2_w[:, None] * fc2_w.T)
    fc2b_eff = fc2_w @ ln2_b + fc2_b
    fc2sum = fc2_t.sum(axis=0)

    use_qk_bias = bool(np.abs(qb).max() > 0 or np.abs(kb).max() > 0)
    use_v_bias = bool(np.abs(vb).max() > 0)
    use_final_affine = not (np.allclose(norm_w, 1.0) and np.allclose(norm_b, 0.0))

    key = (use_v_bias, use_qk_bias, use_final_affine)
    if key not in _compiled:
        _compiled[key] = _build(*key)
    nc = _compiled[key]

    qkvb = np.zeros((P, 3 * NEO), np.float32)
    qkvb[:, 0:NEO] = qb.reshape(NEO, P).T
    qkvb[:, NEO:2 * NEO] = kb.reshape(NEO, P).T
    qkvb[:, 2 * NEO:3 * NEO] = vb.reshape(NEO, P).T
    naff = np.concatenate([norm_w, norm_b]).reshape(1, 2 * D).astype(np.float32)

    common = {
        "wq_t": wq_t.astype(np.float32), "wk_t": wk_t.astype(np.float32),
        "wv_t": wv_t.astype(np.float32),
        "fc1_t": fc1_t.astype(np.float32), "fc2_t": fc2_t.astype(np.float32),
        "fc1b": np.ascontiguousarray(fc1b_eff.reshape(NEO, P).T.astype(np.float32)),
        "fc2sum": fc2sum.reshape(1, D).astype(np.float32),
        "fc2b": fc2b_eff.reshape(1, D).astype(np.float32),
        "qkvb": qkvb, "naff": naff, "vbrow": vb.reshape(1, D).astype(np.float32),
        "onesd": np.full((P, P), 1.0 / D, np.float32),
        "fc1sum": fc1_t.sum(axis=0).reshape(1, D).astype(np.float32),
        "fc1brow": fc1b_eff.reshape(1, D).astype(np.float32),
    }

    in_maps = []
    core_ids = list(range(8))
    for c in core_ids:
        b, half = c // 2, c % 2
        qofs = half * QLOC
        xr = np.roll(x[b], -qofs, axis=0)
        cm = np.roll(np.roll(conn[b, 0], -qofs, axis=0), -qofs, axis=1)
        maskT = cm[:QLOC, :].T.astype(np.float16)            # [A, QLOC]
        maskT = np.ascontiguousarray(
            maskT.reshape(NKT, P, QLOC).transpose(1, 0, 2))          # [P, NKT, QLOC]
        in_maps.append({"x": np.ascontiguousarray(xr), "maskT": maskT, **common})

    res = run_bass_kernel_spmd(nc, in_maps, core_ids)

    y = np.empty((B, A, D), np.float32)
    for c in core_ids:
        b, half = c // 2, c % 2
        y[b, half * QLOC:(half + 1) * QLOC] = res.results[c]["y"]
    return y
